# revision 1
# baseline (speedup 1.0000x reference)
"""AdaGCL encoder (3-hop spmm sum) as a distributed Bass kernel on 8 TRN2 cores.

out = X + A@X + A^2@X + A^3@X, A sparse COO (1.2M edges),
X = concat(user_emb, item_emb) [250000, 64] f32.

Sharding: destination rows are split into 8 blocks of 31250 (one per core).
Each core keeps a full copy of X (input for hop 0; AllGather per hop after),
gathers message sources with `dma_gather` (col-block buckets so indices fit
int16), scales by edge values on the vector engine, and accumulates into
SBUF-resident row accumulators with parity-split `dma_scatter_add` (CCE add
into SBUF — no HBM read-modify-write). Scatter instructions are split into
duplicate-free "layers" (k-th edge of each row) so no two descriptors of one
instruction hit the same row.

Hop outputs are stored in DRAM in the accumulator's natural (permuted) row
order so the SBUF->DRAM dump is fully contiguous; the permutation is baked
into the next hop's gather indices on the host, and undone on the host at
the end. The last hop's accumulator is consumed directly from SBUF by the
final sum.
"""
import numpy as np
import sys

sys.path.insert(0, "/opt/trn_rl_repo")

from concourse import bass, bacc, tile  # noqa: E402
from concourse import mybir  # noqa: E402
from concourse.bass_utils import run_bass_kernel_spmd  # noqa: E402

USER_NUM = 100000
ITEM_NUM = 150000
N = USER_NUM + ITEM_NUM            # 250000
EMB = 64
NB = 8                              # cores == row blocks == col blocks
BLK = N // NB                       # 31250
DUMP = 128                          # dump rows for scatter padding
BLK_PAD = ((BLK + DUMP + 127) // 128) * 128   # 31488
GRP = BLK_PAD // 256                # 123 parity-pair groups
HALF = BLK_PAD // 2                 # 15744 rows per parity half
N_HOPS = 3
F32 = mybir.dt.float32
I16 = mybir.dt.int16

_CACHE = {}


def _round128(x):
    return (int(x) + 127) // 128 * 128


def _pieces(total, maxlen):
    out, s = [], 0
    while s < total:
        L = min(maxlen, total - s)
        out.append((s, L))
        s += L
    return out


def _perm(r):
    """Accumulator layout: local row r=(g*256+t*128+p) -> r'=t*HALF+p*GRP+g."""
    g = r >> 8
    t = (r >> 7) & 1
    p = r & 127
    return t * HALF + p * GRP + g


_PERM = _perm(np.arange(BLK_PAD))          # permuted position of each row


def _preprocess(rows, cols, vals):
    """Per-core, per-bucket padded token streams (see module docstring)."""
    E = len(rows)
    rows = rows.astype(np.int64)
    cols = cols.astype(np.int64)
    core = rows // BLK
    bucket = cols // BLK
    lrow = rows % BLK
    lcol = cols % BLK
    cb = core * NB + bucket

    # sort by (core, bucket, local row); rank = k-th edge of its (cb,row)
    k1 = cb * BLK + lrow
    o1 = np.argsort(k1, kind="stable")
    k1s = k1[o1]
    newgrp = np.r_[True, k1s[1:] != k1s[:-1]]
    gnum = np.cumsum(newgrp) - 1
    gstart = np.flatnonzero(newgrp)
    rank = np.arange(E) - gstart[gnum]
    nL = int(rank.max()) + 1
    cbs = cb[o1]

    cnt = np.zeros((NB * NB, nL), np.int64)
    np.add.at(cnt, (cbs, rank), 1)
    layer_budgets = [_round128(c) for c in cnt.max(axis=0)]
    B = int(sum(layer_budgets))
    layer_off = np.r_[0, np.cumsum(layer_budgets)].astype(np.int64)

    # token position: layer base + running index within (cb, layer)
    k2 = cbs * nL + rank
    o2 = np.argsort(k2, kind="stable")
    k2s = k2[o2]
    newg2 = np.r_[True, k2s[1:] != k2s[:-1]]
    g2num = np.cumsum(newg2) - 1
    g2start = np.flatnonzero(newg2)
    within = np.arange(E) - g2start[g2num]
    pos = layer_off[rank[o2]] + within

    oc = o1[o2]
    g0 = np.zeros((NB, NB, B), np.int16)      # hop-0 gather idx (natural)
    g12 = np.zeros((NB, NB, B), np.int16)     # hop-1/2 gather idx (permuted)
    s = np.tile((BLK + (np.arange(B) % DUMP)).astype(np.int16), (NB, NB, 1))
    v = np.zeros((NB, NB, B), np.float32)
    cc, bb = core[oc], bucket[oc]
    g0[cc, bb, pos] = lcol[oc].astype(np.int16)
    g12[cc, bb, pos] = _PERM[lcol[oc]].astype(np.int16)
    s[cc, bb, pos] = lrow[oc].astype(np.int16)
    v[cc, bb, pos] = vals[oc]

    C16, C128 = B // 16, B // 128
    g0w = np.zeros((NB, NB, 128, C16), np.int16)
    g12w = np.zeros((NB, NB, 128, C16), np.int16)
    sw_all = np.zeros((NB, NB, 128, C16), np.int16)
    vw = np.zeros((NB, NB, 128, C128), np.float32)
    for c in range(NB):
        for b in range(NB):
            a0 = g0[c, b].reshape(C16, 16).T
            a12 = g12[c, b].reshape(C16, 16).T
            vw[c, b] = v[c, b].reshape(C128, 128).T
            swl = np.empty((16, C16), np.int16)
            for li, L in enumerate(layer_budgets):
                t0, t1 = int(layer_off[li]), int(layer_off[li + 1])
                swl[:, t0 // 16:t1 // 16] = \
                    s[c, b, t0:t1].reshape(L // 16, 16).T
            for k in range(8):
                g0w[c, b, 16 * k:16 * k + 16] = a0
                g12w[c, b, 16 * k:16 * k + 16] = a12
                sw_all[c, b, 16 * k:16 * k + 16] = swl
    return g0w, g12w, sw_all, vw, B, layer_budgets


def _build(B, layer_budgets, n_hops=N_HOPS, do_gather=True, do_scatter=True,
           do_collective=True, do_final=True, single_packet=False,
           chunk=4096, chunk_scale=True, schunk=None):
    C16, C128 = B // 16, B // 128
    nc = bacc.Bacc("TRN2", target_bir_lowering=False, debug=False,
                   num_devices=NB)
    x0 = nc.dram_tensor("x0", [N, EMB], F32, kind="ExternalInput")
    x0b = nc.dram_tensor("x0_blk", [BLK_PAD, EMB], F32, kind="ExternalInput")
    gidx0_d = nc.dram_tensor("gidx0", [NB, 128, C16], I16,
                             kind="ExternalInput")
    gidx12_d = nc.dram_tensor("gidx12", [NB, 128, C16], I16,
                              kind="ExternalInput")
    sidx_d = nc.dram_tensor("sidx", [NB, 128, C16], I16, kind="ExternalInput")
    vals_d = nc.dram_tensor("vals", [NB, 128, C128], F32,
                            kind="ExternalInput")
    out = nc.dram_tensor("out", [BLK_PAD, EMB], F32, kind="ExternalOutput")

    HF = HALF * EMB // 128         # 7872 flat f32 per partition per half
    FCH = HF // 12                 # 656

    out_h = [nc.dram_tensor(f"hop_out{k}", [BLK_PAD, EMB], F32)
             for k in range(n_hops)]
    xb = [nc.dram_tensor(f"xgath{k}", [NB * BLK_PAD, EMB], F32)
          for k in range(max(n_hops - 1, 1))]

    with tile.TileContext(nc) as tc:
        with (
            tc.tile_pool(name="meta", bufs=1) as meta,
            tc.tile_pool(name="tok", bufs=2) as tokp,
            tc.tile_pool(name="fin", bufs=3) as finp,
        ):
            gidx_s = meta.tile([128, NB * C16], I16)
            sidx_s = meta.tile([128, NB * C16], I16)
            vals_s = meta.tile([128, NB * C128], F32)
            acc0 = meta.tile([128, GRP, EMB], F32)
            acc1 = meta.tile([128, GRP, EMB], F32)

            for b in range(NB):
                nc.sync.dma_start(gidx_s[:, b * C16:(b + 1) * C16],
                                    gidx0_d[b])
                nc.sync.dma_start(sidx_s[:, b * C16:(b + 1) * C16],
                                    sidx_d[b])
                nc.sync.dma_start(vals_s[:, b * C128:(b + 1) * C128],
                                    vals_d[b])

            for k in range(n_hops):
                outk = out_h[k]
                last = (k == n_hops - 1)
                nc.vector.memset(acc0[:], 0.0)
                nc.vector.memset(acc1[:], 0.0)
                if k == 1:
                    # hop-0 idx no longer needed; swap in the permuted table
                    for b in range(NB):
                        nc.sync.dma_start(
                            gidx_s[:, b * C16:(b + 1) * C16], gidx12_d[b])
                gsrc_s = gidx_s
                for b in range(NB):
                    toks = tokp.tile([128, C128, EMB], F32, tag="toks")
                    if k == 0:
                        src = x0.ap()[b * BLK:(b + 1) * BLK, :]
                    else:
                        src = xb[k - 1].ap()[b * BLK_PAD:(b + 1) * BLK_PAD, :]
                    for s0, L in (_pieces(B, chunk) if do_gather else []):
                        c0, c1 = s0 // 128, (s0 + L) // 128
                        nc.gpsimd.dma_gather(
                            toks[:, c0:c1, :], src,
                            gsrc_s[:, b * C16 + s0 // 16:
                                   b * C16 + (s0 + L) // 16],
                            num_idxs=L, num_idxs_reg=L, elem_size=EMB,
                            single_packet=single_packet)
                        if chunk_scale:
                            nc.vector.tensor_tensor(
                                toks[:, c0:c1, :], toks[:, c0:c1, :],
                                vals_s[:, b * C128 + c0:b * C128 + c1]
                                .unsqueeze(2).broadcast_to(
                                    [128, c1 - c0, EMB]),
                                mybir.AluOpType.mult)
                    if not chunk_scale:
                        nc.vector.tensor_tensor(
                            toks[:], toks[:],
                            vals_s[:, b * C128:(b + 1) * C128]
                            .unsqueeze(2).broadcast_to([128, C128, EMB]),
                            mybir.AluOpType.mult)
                    off = 0
                    for Lb in (layer_budgets if do_scatter else []):
                        for s0, L in _pieces(Lb, schunk or chunk):
                            a = off + s0
                            nc.gpsimd.dma_scatter_add(
                                acc0[:],
                                toks[:, a // 128:(a + L) // 128, :],
                                sidx_s[:, b * C16 + a // 16:
                                       b * C16 + (a + L) // 16],
                                num_idxs=L, num_idxs_reg=L, elem_size=EMB,
                                single_packet=single_packet,
                                sbuf_tokens_per_rank=128,
                                parity_reg=0,
                                out_ap_other=acc1[:])
                        off += Lb
                if not last:
                    # contiguous dump: acc partition p -> rows [p*GRP,(p+1)*GRP)
                    nc.sync.dma_start(
                        outk.ap()[0:HALF, :]
                        .rearrange("(p g) e -> p (g e)", p=128), acc0[:])
                    nc.sync.dma_start(
                        outk.ap()[HALF:BLK_PAD, :]
                        .rearrange("(p g) e -> p (g e)", p=128), acc1[:])
                    if do_collective:
                        nc.gpsimd.collective_compute(
                            "AllGather",
                            mybir.AluOpType.bypass,
                            replica_groups=[list(range(NB))],
                            ins=[outk.ap().opt()],
                            outs=[xb[k].ap().opt()],
                        )

            # out = x0_blk(perm) + hop0 + hop1 + acc(last), two parity halves
            if do_final:
                for half, acct in ((0, acc0), (1, acc1)):
                    rows = slice(half * HALF, (half + 1) * HALF)
                    srcs = [x0b.ap()[rows, :]
                            .rearrange("(p f) e -> p (f e)", p=128)]
                    for k in range(n_hops - 1):
                        srcs.append(out_h[k].ap()[rows, :]
                                    .rearrange("(p f) e -> p (f e)", p=128))
                    dst = out.ap()[rows, :] \
                        .rearrange("(p f) e -> p (f e)", p=128)
                    accf = acct[:].rearrange("p g e -> p (g e)")
                    for j in range(HF // FCH):
                        sl = slice(j * FCH, (j + 1) * FCH)
                        acc = finp.tile([128, FCH], F32, tag="facc")
                        nc.sync.dma_start(acc[:], srcs[0][:, sl])
                        for sf in srcs[1:]:
                            t = finp.tile([128, FCH], F32, tag="fsrc")
                            nc.sync.dma_start(t[:], sf[:, sl])
                            nc.vector.tensor_tensor(acc[:], acc[:], t[:],
                                                    mybir.AluOpType.add)
                        nc.vector.tensor_tensor(acc[:], acc[:], accf[:, sl],
                                                mybir.AluOpType.add)
                        nc.sync.dma_start(dst[:, sl], acc[:])
    nc.compile()
    return nc


def _get_compiled(adj_rows, adj_cols, adj_vals):
    key = (int(adj_rows[0]), int(adj_cols[0]), len(adj_rows))
    if key not in _CACHE:
        g0w, g12w, sw, vw, B, budgets = _preprocess(
            adj_rows, adj_cols, adj_vals)
        nc = _build(B, budgets)
        _CACHE[key] = (nc, g0w, g12w, sw, vw)
    return _CACHE[key]


def kernel(user_emb, item_emb, adj_rows, adj_cols, adj_vals):
    user_emb = np.asarray(user_emb, np.float32)
    item_emb = np.asarray(item_emb, np.float32)
    adj_rows = np.asarray(adj_rows, np.int32)
    adj_cols = np.asarray(adj_cols, np.int32)
    adj_vals = np.asarray(adj_vals, np.float32)

    nc, g0w, g12w, sw, vw = _get_compiled(adj_rows, adj_cols, adj_vals)
    x0 = np.concatenate([user_emb, item_emb], axis=0)

    in_maps = []
    for c in range(NB):
        xp = np.zeros((BLK_PAD, EMB), np.float32)
        xp[_PERM[:BLK]] = x0[c * BLK:(c + 1) * BLK]
        in_maps.append({
            "x0": x0,
            "x0_blk": xp,
            "gidx0": g0w[c],
            "gidx12": g12w[c],
            "sidx": sw[c],
            "vals": vw[c],
        })
    res = run_bass_kernel_spmd(nc, in_maps, core_ids=list(range(NB)))
    blocks = [res.results[c]["out"][_PERM[:BLK]] for c in range(NB)]
    return np.concatenate(blocks, axis=0)



# revision 5
# speedup vs baseline: 1.1894x; 1.1894x over previous
"""AdaGCL encoder (3-hop spmm sum) as a distributed Bass kernel on 8 TRN2 cores.

out = X + A@X + A^2@X + A^3@X, A sparse COO (1.2M edges),
X = concat(user_emb, item_emb) [250000, 64] f32.

Sharding: destination rows are split into 8 blocks of 31250 (one per core).
Each core keeps a full copy of X (input for hop 0; AllGather per hop after),
gathers message sources with `dma_gather` (col-block buckets so indices fit
int16), scales by edge values on the vector engine, and accumulates into
SBUF-resident row accumulators with parity-split `dma_scatter_add` (CCE add
into SBUF — no HBM read-modify-write). Scatter instructions are split into
duplicate-free "layers" (k-th edge of each row) so no two descriptors of one
instruction hit the same row.

Hop outputs are stored in DRAM in the accumulator's natural (permuted) row
order so the SBUF->DRAM dump is fully contiguous; the permutation is baked
into the next hop's gather indices on the host, and undone on the host at
the end. The last hop's accumulator is consumed directly from SBUF by the
final sum.
"""
import numpy as np
import sys

sys.path.insert(0, "/opt/trn_rl_repo")

from concourse import bass, bacc, tile  # noqa: E402
from concourse import mybir  # noqa: E402
from concourse.bass_utils import run_bass_kernel_spmd  # noqa: E402

USER_NUM = 100000
ITEM_NUM = 150000
N = USER_NUM + ITEM_NUM            # 250000
EMB = 64
NB = 8                              # cores == row blocks == col blocks
BLK = N // NB                       # 31250
DUMP = 128                          # dump rows for scatter padding
BLK_PAD = ((BLK + DUMP + 127) // 128) * 128   # 31488
GRP = BLK_PAD // 256                # 123 parity-pair groups
HALF = BLK_PAD // 2                 # 15744 rows per parity half
N_HOPS = 3
F32 = mybir.dt.float32
I16 = mybir.dt.int16

_CACHE = {}


def _round128(x):
    return (int(x) + 127) // 128 * 128


def _pieces(total, maxlen):
    out, s = [], 0
    while s < total:
        L = min(maxlen, total - s)
        out.append((s, L))
        s += L
    return out


def _perm(r):
    """Accumulator layout: local row r=(g*256+t*128+p) -> r'=t*HALF+p*GRP+g."""
    g = r >> 8
    t = (r >> 7) & 1
    p = r & 127
    return t * HALF + p * GRP + g


_PERM = _perm(np.arange(BLK_PAD))          # permuted position of each row


def _preprocess(rows, cols, vals):
    """Per-core, per-bucket padded token streams (see module docstring)."""
    E = len(rows)
    rows = rows.astype(np.int64)
    cols = cols.astype(np.int64)
    core = rows // BLK
    bucket = cols // BLK
    lrow = rows % BLK
    lcol = cols % BLK
    cb = core * NB + bucket

    # sort by (core, bucket, local row); rank = k-th edge of its (cb,row)
    k1 = cb * BLK + lrow
    o1 = np.argsort(k1, kind="stable")
    k1s = k1[o1]
    newgrp = np.r_[True, k1s[1:] != k1s[:-1]]
    gnum = np.cumsum(newgrp) - 1
    gstart = np.flatnonzero(newgrp)
    rank = np.arange(E) - gstart[gnum]
    nL = int(rank.max()) + 1
    cbs = cb[o1]

    cnt = np.zeros((NB * NB, nL), np.int64)
    np.add.at(cnt, (cbs, rank), 1)
    layer_budgets = [_round128(c) for c in cnt.max(axis=0)]
    B = int(sum(layer_budgets))
    layer_off = np.r_[0, np.cumsum(layer_budgets)].astype(np.int64)

    # token position: layer base + running index within (cb, layer)
    k2 = cbs * nL + rank
    o2 = np.argsort(k2, kind="stable")
    k2s = k2[o2]
    newg2 = np.r_[True, k2s[1:] != k2s[:-1]]
    g2num = np.cumsum(newg2) - 1
    g2start = np.flatnonzero(newg2)
    within = np.arange(E) - g2start[g2num]
    pos = layer_off[rank[o2]] + within

    oc = o1[o2]
    g0 = np.zeros((NB, NB, B), np.int16)      # hop-0 gather idx (natural)
    g12 = np.zeros((NB, NB, B), np.int16)     # hop-1/2 gather idx (permuted)
    s = np.tile((BLK + (np.arange(B) % DUMP)).astype(np.int16), (NB, NB, 1))
    v = np.zeros((NB, NB, B), np.float32)
    cc, bb = core[oc], bucket[oc]
    g0[cc, bb, pos] = lcol[oc].astype(np.int16)
    g12[cc, bb, pos] = _PERM[lcol[oc]].astype(np.int16)
    s[cc, bb, pos] = lrow[oc].astype(np.int16)
    v[cc, bb, pos] = vals[oc]

    C16, C128 = B // 16, B // 128
    g0w = np.zeros((NB, NB, 128, C16), np.int16)
    g12w = np.zeros((NB, NB, 128, C16), np.int16)
    sw_all = np.zeros((NB, NB, 128, C16), np.int16)
    vw = np.zeros((NB, NB, 128, C128), np.float32)
    for c in range(NB):
        for b in range(NB):
            a0 = g0[c, b].reshape(C16, 16).T
            a12 = g12[c, b].reshape(C16, 16).T
            vw[c, b] = v[c, b].reshape(C128, 128).T
            swl = np.empty((16, C16), np.int16)
            for li, L in enumerate(layer_budgets):
                t0, t1 = int(layer_off[li]), int(layer_off[li + 1])
                swl[:, t0 // 16:t1 // 16] = \
                    s[c, b, t0:t1].reshape(L // 16, 16).T
            for k in range(8):
                g0w[c, b, 16 * k:16 * k + 16] = a0
                g12w[c, b, 16 * k:16 * k + 16] = a12
                sw_all[c, b, 16 * k:16 * k + 16] = swl
    return g0w, g12w, sw_all, vw, B, layer_budgets


def _build(B, layer_budgets, n_hops=N_HOPS, do_gather=True, do_scatter=True,
           do_collective=True, do_final=True, single_packet=False,
           chunk=4096, chunk_scale=True, schunk=None,
           nq=1, gq=0, sq=0, shared_xb=False):
    C16, C128 = B // 16, B // 128
    nc = bacc.Bacc("TRN2", target_bir_lowering=False, debug=False,
                   num_devices=NB, num_swdge_queues=nq)
    x0 = nc.dram_tensor("x0", [N, EMB], F32, kind="ExternalInput")
    x0b = nc.dram_tensor("x0_blk", [BLK_PAD, EMB], F32, kind="ExternalInput")
    gidx0_d = nc.dram_tensor("gidx0", [NB, 128, C16], I16,
                             kind="ExternalInput")
    gidx12_d = nc.dram_tensor("gidx12", [NB, 128, C16], I16,
                              kind="ExternalInput")
    sidx_d = nc.dram_tensor("sidx", [NB, 128, C16], I16, kind="ExternalInput")
    vals_d = nc.dram_tensor("vals", [NB, 128, C128], F32,
                            kind="ExternalInput")
    out = nc.dram_tensor("out", [BLK_PAD, EMB], F32, kind="ExternalOutput")

    HF = HALF * EMB // 128         # 7872 flat f32 per partition per half
    FCH = HF // 12                 # 656

    out_h = [nc.dram_tensor(f"hop_out{k}", [BLK_PAD, EMB], F32)
             for k in range(n_hops)]
    xb = [nc.dram_tensor(f"xgath{k}", [NB * BLK_PAD, EMB], F32,
                         addr_space="Shared" if shared_xb else "Local")
          for k in range(max(n_hops - 1, 1))]

    with tile.TileContext(nc) as tc:
        with (
            tc.tile_pool(name="meta", bufs=1) as meta,
            tc.tile_pool(name="tok", bufs=2) as tokp,
            tc.tile_pool(name="fin", bufs=3) as finp,
        ):
            gidx_s = meta.tile([128, NB * C16], I16)
            sidx_s = meta.tile([128, NB * C16], I16)
            vals_s = meta.tile([128, NB * C128], F32)
            acc0 = meta.tile([128, GRP, EMB], F32)
            acc1 = meta.tile([128, GRP, EMB], F32)

            for b in range(NB):
                nc.sync.dma_start(gidx_s[:, b * C16:(b + 1) * C16],
                                    gidx0_d[b])
                nc.sync.dma_start(sidx_s[:, b * C16:(b + 1) * C16],
                                    sidx_d[b])
                nc.sync.dma_start(vals_s[:, b * C128:(b + 1) * C128],
                                    vals_d[b])

            for k in range(n_hops):
                outk = out_h[k]
                last = (k == n_hops - 1)
                nc.vector.memset(acc0[:], 0.0)
                nc.vector.memset(acc1[:], 0.0)
                if k == 1:
                    # hop-0 idx no longer needed; swap in the permuted table
                    for b in range(NB):
                        nc.sync.dma_start(
                            gidx_s[:, b * C16:(b + 1) * C16], gidx12_d[b])
                gsrc_s = gidx_s
                for b in range(NB):
                    toks = tokp.tile([128, C128, EMB], F32, tag="toks")
                    if k == 0:
                        src = x0.ap()[b * BLK:(b + 1) * BLK, :]
                    else:
                        src = xb[k - 1].ap()[b * BLK_PAD:(b + 1) * BLK_PAD, :]
                    for s0, L in (_pieces(B, chunk) if do_gather else []):
                        c0, c1 = s0 // 128, (s0 + L) // 128
                        nc.gpsimd.dma_gather(
                            toks[:, c0:c1, :], src,
                            gsrc_s[:, b * C16 + s0 // 16:
                                   b * C16 + (s0 + L) // 16],
                            num_idxs=L, num_idxs_reg=L, elem_size=EMB,
                            single_packet=single_packet, queue_num=gq)
                        if chunk_scale:
                            nc.vector.tensor_tensor(
                                toks[:, c0:c1, :], toks[:, c0:c1, :],
                                vals_s[:, b * C128 + c0:b * C128 + c1]
                                .unsqueeze(2).broadcast_to(
                                    [128, c1 - c0, EMB]),
                                mybir.AluOpType.mult)
                    if not chunk_scale:
                        nc.vector.tensor_tensor(
                            toks[:], toks[:],
                            vals_s[:, b * C128:(b + 1) * C128]
                            .unsqueeze(2).broadcast_to([128, C128, EMB]),
                            mybir.AluOpType.mult)
                    off = 0
                    for Lb in (layer_budgets if do_scatter else []):
                        for s0, L in _pieces(Lb, schunk or chunk):
                            a = off + s0
                            nc.gpsimd.dma_scatter_add(
                                acc0[:],
                                toks[:, a // 128:(a + L) // 128, :],
                                sidx_s[:, b * C16 + a // 16:
                                       b * C16 + (a + L) // 16],
                                num_idxs=L, num_idxs_reg=L, elem_size=EMB,
                                single_packet=single_packet, queue_num=sq,
                                sbuf_tokens_per_rank=128,
                                parity_reg=0,
                                out_ap_other=acc1[:])
                        off += Lb
                if not last:
                    # contiguous dump: acc partition p -> rows [p*GRP,(p+1)*GRP)
                    nc.sync.dma_start(
                        outk.ap()[0:HALF, :]
                        .rearrange("(p g) e -> p (g e)", p=128), acc0[:])
                    nc.sync.dma_start(
                        outk.ap()[HALF:BLK_PAD, :]
                        .rearrange("(p g) e -> p (g e)", p=128), acc1[:])
                    if do_collective:
                        nc.gpsimd.collective_compute(
                            "AllGather",
                            mybir.AluOpType.bypass,
                            replica_groups=[list(range(NB))],
                            ins=[outk.ap().opt()],
                            outs=[xb[k].ap().opt()],
                        )

            # out = x0_blk(perm) + hop0 + hop1 + acc(last), two parity halves
            if do_final:
                for half, acct in ((0, acc0), (1, acc1)):
                    rows = slice(half * HALF, (half + 1) * HALF)
                    srcs = [x0b.ap()[rows, :]
                            .rearrange("(p f) e -> p (f e)", p=128)]
                    for k in range(n_hops - 1):
                        srcs.append(out_h[k].ap()[rows, :]
                                    .rearrange("(p f) e -> p (f e)", p=128))
                    dst = out.ap()[rows, :] \
                        .rearrange("(p f) e -> p (f e)", p=128)
                    accf = acct[:].rearrange("p g e -> p (g e)")
                    for j in range(HF // FCH):
                        sl = slice(j * FCH, (j + 1) * FCH)
                        acc = finp.tile([128, FCH], F32, tag="facc")
                        nc.sync.dma_start(acc[:], srcs[0][:, sl])
                        for sf in srcs[1:]:
                            t = finp.tile([128, FCH], F32, tag="fsrc")
                            nc.sync.dma_start(t[:], sf[:, sl])
                            nc.vector.tensor_tensor(acc[:], acc[:], t[:],
                                                    mybir.AluOpType.add)
                        nc.vector.tensor_tensor(acc[:], acc[:], accf[:, sl],
                                                mybir.AluOpType.add)
                        nc.sync.dma_start(dst[:, sl], acc[:])
    nc.compile()
    return nc


def _get_compiled(adj_rows, adj_cols, adj_vals):
    key = (int(adj_rows[0]), int(adj_cols[0]), len(adj_rows))
    if key not in _CACHE:
        g0w, g12w, sw, vw, B, budgets = _preprocess(
            adj_rows, adj_cols, adj_vals)
        nc = _build(B, budgets)
        _CACHE[key] = (nc, g0w, g12w, sw, vw)
    return _CACHE[key]


def kernel(user_emb, item_emb, adj_rows, adj_cols, adj_vals):
    user_emb = np.asarray(user_emb, np.float32)
    item_emb = np.asarray(item_emb, np.float32)
    adj_rows = np.asarray(adj_rows, np.int32)
    adj_cols = np.asarray(adj_cols, np.int32)
    adj_vals = np.asarray(adj_vals, np.float32)

    nc, g0w, g12w, sw, vw = _get_compiled(adj_rows, adj_cols, adj_vals)
    x0 = np.concatenate([user_emb, item_emb], axis=0)

    in_maps = []
    for c in range(NB):
        xp = np.zeros((BLK_PAD, EMB), np.float32)
        xp[_PERM[:BLK]] = x0[c * BLK:(c + 1) * BLK]
        in_maps.append({
            "x0": x0,
            "x0_blk": xp,
            "gidx0": g0w[c],
            "gidx12": g12w[c],
            "sidx": sw[c],
            "vals": vw[c],
        })
    res = run_bass_kernel_spmd(nc, in_maps, core_ids=list(range(NB)))
    blocks = [res.results[c]["out"][_PERM[:BLK]] for c in range(NB)]
    return np.concatenate(blocks, axis=0)

